# revision 10
# baseline (speedup 1.0000x reference)
"""Two-layer GraphSAGE (mean aggr) + log_softmax on 8 TRN2 NeuronCores.

Strategy
--------
Destination nodes are partitioned across the 8 cores (12544 nodes each, padded
from 100000 to 100352); each core owns the edges that TARGET its node shard.
Node features are replicated in HBM, so the per-edge gather x[src] is local
(dma_gather, int16 indices, 4 row-buckets of <=32768 rows, striped over the 4
SWDGE queues — the gather is HBM-random-read-latency bound, so queue count is
the main throughput lever). The mean segment-reduction runs on the
TensorEngine: per 128-edge chunk, a 0/1 one-hot S[e,d] = (dst_local_e == d) is
matmul-accumulated into PSUM ([dst, feat]); the 1/deg scaling is a
per-partition tensor_scalar on the PSUM->SBUF copy. One-hots for all chunks of
a tile are built in ONE DVE tensor_tensor(is_equal) using step-0 broadcast
access patterns. Layer 2 pre-transforms h1 @ Wl2 into z per shard, AllGathers
z ([100352,128] bf16), and repeats the same gather+segment-matmul with the
same index structure.

Edges are laid out [wave][bucket][tile][128-slot chunks] so a handful of large
gathers per (wave, bucket) cover 7 destination tiles. All 8 cores share one
BIR, so chunk counts per (tile, bucket) are the max across cores.
"""

import sys
import types

import numpy as np
import ml_dtypes

import concourse.bass as bass
import concourse.bacc as bacc
import concourse.tile as tile
import concourse.mybir as mybir
from concourse.bass_utils import run_bass_kernel_spmd

NBF = ml_dtypes.bfloat16
BF16 = mybir.dt.bfloat16
F32 = mybir.dt.float32
I16 = mybir.dt.int16

N = 100000
E = 1600000
F = 128
H = 128
O = 40

NCORES = 8
TILE = 128
NT = 98                 # dst tiles per core
PER = NT * TILE         # 12544 dst nodes per core
NPAD = NCORES * PER     # 100352
WAVE_T = 7              # dst tiles per wave
NW = NT // WAVE_T       # 14 waves
BUCK = 32768
NB = 4                  # src-index buckets (int16 gather indices)
BROWS = [BUCK, BUCK, BUCK, NPAD - 3 * BUCK]
GPIECE = 2048           # idxs per gather piece (round-robined over 4 queues)
DLPAD = 200.0           # dst_local pad value (never matches iota 0..127)


def _install_ntff_shim():
    """antenv.axon_hooks is missing in this image; bass_utils trace=True needs it."""
    if "antenv.axon_hooks" in sys.modules:
        return
    mod = types.ModuleType("antenv.axon_hooks")
    mod._hook = None
    mod.set_axon_ntff_profile_hook = lambda h: setattr(mod, "_hook", h)
    mod.get_axon_ntff_profile_hook = lambda: mod._hook
    sys.modules["antenv.axon_hooks"] = mod
    try:
        import antenv

        antenv.axon_hooks = mod
        from trn_agent_boot.trn_boot import _ntff_profile_via_ctypes

        mod._hook = _ntff_profile_via_ctypes("/opt/axon/libaxon_pjrt.so")
    except Exception:
        pass


def _prep(x, edge_index, Wl1, b1, Wr1, Wl2, b2, Wr2):
    """Host-side sharding: slot layout, gather indices, one-hot operands."""
    x = np.asarray(x, np.float32)
    src = np.asarray(edge_index[0], np.int64)
    dst = np.asarray(edge_index[1], np.int64)

    deg = np.bincount(dst, minlength=NPAD)
    w_node = (1.0 / np.maximum(deg, 1)).astype(np.float32)  # [NPAD]

    xtab = np.zeros((NPAD, F), NBF)
    xtab[:N] = x.astype(NBF)

    # ---- per-core edge grouping ----
    NG = NW * NB * NT
    per_core = []
    counts = np.zeros((NCORES, NG), np.int64)
    for c in range(NCORES):
        lo = c * PER
        m = (dst >= lo) & (dst < lo + PER)
        s = src[m]
        d = dst[m] - lo
        t_id = d >> 7
        b_id = np.minimum(s >> 15, NB - 1)
        wv = t_id // WAVE_T
        g = (wv * NB + b_id) * NT + t_id
        o = np.argsort(g, kind="stable")
        s, d, g = s[o], d[o], g[o]
        counts[c] = np.bincount(g, minlength=NG)
        per_core.append((s, d, g))

    # uniform (shared across cores) slot structure
    slots_g = ((counts.max(axis=0) + TILE - 1) // TILE) * TILE
    off_g = np.zeros(NG + 1, np.int64)
    np.cumsum(slots_g, out=off_g[1:])
    S_TOT = int(off_g[-1])
    TC = S_TOT // TILE  # total chunks

    # per-wave / per-(wave,bucket) / per-(tile,bucket) static offsets
    wave_off = [int(off_g[(wv * NB) * NT]) for wv in range(NW)] + [S_TOT]
    gather_meta = []  # (wv, slot_start, n_idxs, bucket)
    for wv in range(NW):
        for b in range(NB):
            g0 = (wv * NB + b) * NT
            start = int(off_g[g0])
            end = int(off_g[g0 + NT])
            pos = start
            while pos < end:
                n = min(GPIECE, end - pos)
                gather_meta.append((wv, pos, n, b))
                pos += n

    # per tile: list of (wave_block, slot_chunk); one-hot columns are
    # tile-major (column = running count in this iteration order)
    tile_chunks = []
    for t in range(NT):
        wv = t // WAVE_T
        lst = []
        for b in range(NB):
            g = (wv * NB + b) * NT + t
            st, sl = int(off_g[g]), int(slots_g[g])
            for j in range(sl // TILE):
                blk = (st - wave_off[wv]) // TILE + j
                lst.append((blk, st // TILE + j))
        tile_chunks.append(lst)

    # ---- per-core input arrays ----
    in_maps = []
    wl1 = np.asarray(Wl1, np.float32).astype(NBF)
    wr1 = np.asarray(Wr1, np.float32).astype(NBF)
    wl2p = np.zeros((H, 128), NBF)
    wl2p[:, :O] = np.asarray(Wl2, np.float32).astype(NBF)
    wr2 = np.asarray(Wr2, np.float32).astype(NBF)
    b1c = np.asarray(b1, np.float32).reshape(H, 1)
    b2r = np.asarray(b2, np.float32).astype(NBF).reshape(1, O)
    ones_r = np.ones((1, 128), NBF)
    iota = np.tile(np.arange(128, dtype=np.float32), (128, 1)).astype(NBF)
    ident = np.eye(128, dtype=np.float32).astype(NBF)

    # slot-chunk -> tile-major one-hot column
    col_of_chunk = np.full(TC, -1, np.int64)
    ncol = 0
    for t in range(NT):
        for (_, sc) in tile_chunks[t]:
            col_of_chunk[sc] = ncol
            ncol += 1
    assert ncol == TC

    for c in range(NCORES):
        s, d, g = per_core[c]
        first = np.zeros(NG + 1, np.int64)
        np.cumsum(counts[c], out=first[1:])
        rank = np.arange(len(s)) - first[g]
        slot = off_g[g] + rank

        gidx_flat = np.zeros(S_TOT, np.int16)
        gidx_flat[slot] = (s - np.minimum(s >> 15, NB - 1) * BUCK).astype(np.int16)
        dl_flat = np.full(S_TOT, DLPAD, np.float32)
        dl_flat[slot] = (d & 127).astype(np.float32)

        # [128, TC] slot-chunk layout -> permute columns to tile-major
        dl_cols = dl_flat.reshape(-1, TILE).T
        dlT = np.empty_like(dl_cols)
        dlT[:, col_of_chunk] = dl_cols

        lo = c * PER
        wc = w_node[lo : lo + PER].reshape(NT, TILE).T  # [128, NT]

        xT = np.zeros((F, PER), NBF)
        n_real = min(N - lo, PER)
        xT[:, :n_real] = x[lo : lo + n_real].T.astype(NBF)

        in_maps.append(
            {
                "xtab": xtab,
                "xT": np.ascontiguousarray(xT),
                "gidx": np.ascontiguousarray(np.tile(gidx_flat.reshape(-1, 16).T, (8, 1))),
                "dl": np.ascontiguousarray(dlT.astype(NBF)),
                "wc": np.ascontiguousarray(wc.astype(np.float32)),
                "wl1": wl1,
                "wr1": wr1,
                "wl2p": wl2p,
                "wr2": wr2,
                "b1": b1c,
                "b2r": b2r,
                "ones": ones_r,
                "iota": iota,
                "ident": ident,
            }
        )

    meta = dict(S_TOT=S_TOT, TC=TC, wave_off=wave_off, gather_meta=gather_meta,
                tile_chunks=tile_chunks)
    return in_maps, meta


def _build(meta):
    S_TOT = meta["S_TOT"]
    TC = meta["TC"]
    wave_off = meta["wave_off"]
    gather_meta = meta["gather_meta"]
    tile_chunks = meta["tile_chunks"]
    max_nch = max(len(ch) for ch in tile_chunks)

    nc = bacc.Bacc(num_devices=NCORES, num_swdge_queues=4)

    d_xtab = nc.declare_dram_parameter("xtab", [NPAD, F], BF16, isOutput=False)
    d_xT = nc.declare_dram_parameter("xT", [F, PER], BF16, isOutput=False)
    d_gidx = nc.declare_dram_parameter("gidx", [128, S_TOT // 16], I16, isOutput=False)
    d_dl = nc.declare_dram_parameter("dl", [128, TC], BF16, isOutput=False)
    d_wc = nc.declare_dram_parameter("wc", [128, NT], F32, isOutput=False)
    d_wl1 = nc.declare_dram_parameter("wl1", [F, H], BF16, isOutput=False)
    d_wr1 = nc.declare_dram_parameter("wr1", [F, H], BF16, isOutput=False)
    d_wl2p = nc.declare_dram_parameter("wl2p", [H, 128], BF16, isOutput=False)
    d_wr2 = nc.declare_dram_parameter("wr2", [H, O], BF16, isOutput=False)
    d_b1 = nc.declare_dram_parameter("b1", [H, 1], F32, isOutput=False)
    d_b2r = nc.declare_dram_parameter("b2r", [1, O], BF16, isOutput=False)
    d_ones = nc.declare_dram_parameter("ones", [1, 128], BF16, isOutput=False)
    d_iota = nc.declare_dram_parameter("iota", [128, 128], BF16, isOutput=False)
    d_ident = nc.declare_dram_parameter("ident", [128, 128], BF16, isOutput=False)
    d_out = nc.declare_dram_parameter("out", [PER, O], F32, isOutput=True)

    z_shard = nc.dram_tensor("z_shard", [PER, 128], BF16, kind="Internal")
    z_full = nc.dram_tensor("z_full", [NPAD, 128], BF16, kind="Internal",
                            addr_space="Shared")

    AOT = mybir.AluOpType
    AFT = mybir.ActivationFunctionType

    with tile.TileContext(nc) as tc:
        with (
            tc.tile_pool(name="const", bufs=1) as cpool,
            tc.tile_pool(name="gpool", bufs=2) as gpool,
            tc.tile_pool(name="spool", bufs=3) as spool,
            tc.tile_pool(name="opool", bufs=3) as opool,
            tc.tile_pool(name="ppA", bufs=2, space="PSUM") as ppA,
            tc.tile_pool(name="ppT", bufs=2, space="PSUM") as ppT,
            tc.tile_pool(name="ppB", bufs=2, space="PSUM") as ppB,
        ):
            gidx_sb = cpool.tile([128, S_TOT // 16], I16)
            nc.sync.dma_start(gidx_sb[:], d_gidx[:])
            dl_sb = cpool.tile([128, TC], BF16)
            nc.sync.dma_start(dl_sb[:], d_dl[:])
            wc_sb = cpool.tile([128, NT], F32)
            nc.sync.dma_start(wc_sb[:], d_wc[:])
            iota_sb = cpool.tile([128, 128], BF16)
            nc.sync.dma_start(iota_sb[:], d_iota[:])
            ident_sb = cpool.tile([128, 128], BF16)
            nc.sync.dma_start(ident_sb[:], d_ident[:])
            xT_sb = cpool.tile([F, PER], BF16)
            nc.sync.dma_start(xT_sb[:], d_xT[:])
            wl1_sb = cpool.tile([F, H], BF16)
            nc.sync.dma_start(wl1_sb[:], d_wl1[:])
            wr1_sb = cpool.tile([F, H], BF16)
            nc.sync.dma_start(wr1_sb[:], d_wr1[:])
            wl2_sb = cpool.tile([H, 128], BF16)
            nc.sync.dma_start(wl2_sb[:], d_wl2p[:])
            wr2_sb = cpool.tile([H, O], BF16)
            nc.sync.dma_start(wr2_sb[:], d_wr2[:])
            b1_sb = cpool.tile([H, 1], F32)
            nc.sync.dma_start(b1_sb[:], d_b1[:])
            b2_sb = cpool.tile([1, O], BF16)
            nc.sync.dma_start(b2_sb[:], d_b2r[:])
            ones_sb = cpool.tile([1, 128], BF16)
            nc.sync.dma_start(ones_sb[:], d_ones[:])
            r2_all = cpool.tile([128, NT * O], F32)

            qctr = [0]

            def gathers_for_wave(wv, table_slices, Gt):
                for (w2, start, n, b) in gather_meta:
                    if w2 != wv:
                        continue
                    blk0 = (start - wave_off[wv]) // TILE
                    nc.gpsimd.dma_gather(
                        out_ap=Gt[:, blk0 : blk0 + n // TILE, :],
                        in_ap=table_slices[b],
                        idxs_ap=gidx_sb[:, start // 16 : (start + n) // 16],
                        num_idxs=n,
                        num_idxs_reg=n,
                        elem_size=128,
                        single_packet=False,
                        queue_num=qctr[0] % 4,
                    )
                    qctr[0] += 1

            def onehot_tile(t):
                """Raw 0/1 one-hots for all chunks of tile t: [128, nch, 128]."""
                nch = len(tile_chunks[t])
                col0 = col_base[t]
                S_t = spool.tile([128, max_nch, 128], BF16, tag="S")
                it_b = iota_sb[:].unsqueeze(1).broadcast_to([128, nch, 128])
                dl_b = dl_sb[:, col0 : col0 + nch].unsqueeze(2).broadcast_to(
                    [128, nch, 128]
                )
                nc.vector.tensor_tensor(S_t[:, :nch, :], it_b, dl_b, AOT.is_equal)
                return S_t

            col_base = []
            acc = 0
            for t in range(NT):
                col_base.append(acc)
                acc += len(tile_chunks[t])

            xslices = [d_xtab[sum(BROWS[:b]) : sum(BROWS[: b + 1]), :] for b in range(NB)]
            zslices = [z_full[sum(BROWS[:b]) : sum(BROWS[: b + 1]), :] for b in range(NB)]

            # ---------------- phase 1: layer 1 + z/r2 ----------------
            for wv in range(NW):
                nc.enter_named_scope(f"p1w{wv:02d}", False)
                nblk = (wave_off[wv + 1] - wave_off[wv]) // TILE
                Gt = gpool.tile([128, nblk, 128], BF16, tag="G")
                gathers_for_wave(wv, xslices, Gt)
                for t in range(wv * WAVE_T, (wv + 1) * WAVE_T):
                    chunks = tile_chunks[t]
                    pH = ppB.tile([128, 128], F32, tag="pH")
                    if chunks:
                        S_t = onehot_tile(t)
                        pA = ppA.tile([128, 128], F32, tag="pA")
                        for k, (blk, _) in enumerate(chunks):
                            nc.tensor.matmul(
                                pA[:], S_t[:, k, :], Gt[:, blk, :],
                                start=(k == 0), stop=(k == len(chunks) - 1),
                            )
                        aggr = spool.tile([128, 128], BF16, tag="aggr")
                        nc.vector.tensor_scalar(
                            aggr[:], pA[:], wc_sb[:, t : t + 1], None, AOT.mult
                        )
                        pT = ppT.tile([128, 128], BF16, tag="pT")
                        nc.tensor.transpose(pT[:], aggr[:], ident_sb[:])
                        aggrT = spool.tile([128, 128], BF16, tag="aggrT")
                        nc.vector.tensor_copy(aggrT[:], pT[:])
                        nc.tensor.matmul(pH[:], wl1_sb[:], aggrT[:], start=True, stop=False)
                        nc.tensor.matmul(
                            pH[:], wr1_sb[:], xT_sb[:, t * TILE : (t + 1) * TILE],
                            start=False, stop=True,
                        )
                    else:
                        nc.tensor.matmul(
                            pH[:], wr1_sb[:], xT_sb[:, t * TILE : (t + 1) * TILE],
                            start=True, stop=True,
                        )
                    h1T = spool.tile([128, 128], BF16, tag="h1T")
                    nc.scalar.activation(h1T[:], pH[:], AFT.Relu, bias=b1_sb[:])
                    pZ = ppB.tile([128, 128], F32, tag="pZR")
                    nc.tensor.matmul(pZ[:], h1T[:], wl2_sb[:], start=True, stop=True)
                    zt = spool.tile([128, 128], BF16, tag="zt")
                    nc.vector.tensor_copy(zt[:], pZ[:])
                    nc.sync.dma_start(z_shard[t * TILE : (t + 1) * TILE, :], zt[:])
                    pR = ppB.tile([128, O], F32, tag="pZR")
                    nc.tensor.matmul(pR[:], h1T[:], wr2_sb[:], start=True, stop=False)
                    nc.tensor.matmul(pR[:], ones_sb[:], b2_sb[:], start=False, stop=True)
                    nc.vector.tensor_copy(r2_all[:, t * O : (t + 1) * O], pR[:])
                nc.leave_named_scope(f"p1w{wv:02d}", None, False)

            # ---------------- z all-gather ----------------
            nc.enter_named_scope("allgather", False)
            tc.strict_bb_all_engine_barrier()
            nc.gpsimd.collective_compute(
                "AllGather",
                AOT.bypass,
                replica_groups=[list(range(NCORES))],
                ins=[z_shard[:]],
                outs=[z_full[:]],
            )
            tc.strict_bb_all_engine_barrier()
            nc.leave_named_scope("allgather", None, False)

            # ---------------- phase 2: layer 2 + log_softmax ----------------
            for wv in range(NW):
                nc.enter_named_scope(f"p2w{wv:02d}", False)
                nblk = (wave_off[wv + 1] - wave_off[wv]) // TILE
                Gt = gpool.tile([128, nblk, 128], BF16, tag="G")
                gathers_for_wave(wv, zslices, Gt)
                for t in range(wv * WAVE_T, (wv + 1) * WAVE_T):
                    chunks = tile_chunks[t]
                    tsb = opool.tile([128, O], F32, tag="tsb")
                    if chunks:
                        S_t = onehot_tile(t)
                        pO = ppA.tile([128, 64], F32, tag="pA")
                        for k, (blk, _) in enumerate(chunks):
                            nc.tensor.matmul(
                                pO[:], S_t[:, k, :], Gt[:, blk, 0:64],
                                start=(k == 0), stop=(k == len(chunks) - 1),
                            )
                        m2 = opool.tile([128, O], F32, tag="m2")
                        nc.vector.tensor_scalar(
                            m2[:], pO[:, :O], wc_sb[:, t : t + 1], None, AOT.mult
                        )
                        nc.vector.tensor_tensor(
                            tsb[:], m2[:], r2_all[:, t * O : (t + 1) * O], AOT.add
                        )
                    else:
                        nc.vector.tensor_copy(tsb[:], r2_all[:, t * O : (t + 1) * O])
                    nmax = opool.tile([128, 1], F32, tag="nmax")
                    nc.vector.tensor_reduce(
                        out=nmax[:], in_=tsb[:], op=AOT.max,
                        axis=mybir.AxisListType.X, negate=True,
                    )
                    esb = opool.tile([128, O], F32, tag="esb")
                    sumx = opool.tile([128, 1], F32, tag="sumx")
                    nc.scalar.activation(esb[:], tsb[:], AFT.Exp, bias=nmax[:],
                                         accum_out=sumx[:])
                    lse = opool.tile([128, 1], F32, tag="lse")
                    nc.scalar.activation(lse[:], sumx[:], AFT.Ln)
                    shift = opool.tile([128, 1], F32, tag="shift")
                    nc.vector.tensor_tensor(shift[:], nmax[:], lse[:], AOT.subtract)
                    osb = opool.tile([128, O], F32, tag="osb")
                    nc.vector.tensor_scalar(osb[:], tsb[:], shift[:], None, AOT.add)
                    nc.sync.dma_start(d_out[t * TILE : (t + 1) * TILE, :], osb[:])
                nc.leave_named_scope(f"p2w{wv:02d}", None, False)

    nc.compile()
    return nc


def _run(inputs, trace=False):
    _install_ntff_shim()
    in_maps, meta = _prep(
        inputs["x"], inputs["edge_index"], inputs["Wl1"], inputs["b1"],
        inputs["Wr1"], inputs["Wl2"], inputs["b2"], inputs["Wr2"],
    )
    nc = _build(meta)
    res = run_bass_kernel_spmd(nc, in_maps, core_ids=list(range(NCORES)), trace=trace)
    out = np.concatenate([res.results[c]["out"] for c in range(NCORES)], axis=0)[:N]
    return np.ascontiguousarray(out), res


def kernel(**inputs):
    out, _ = _run(inputs, trace=False)
    return out
